# revision 9
# baseline (speedup 1.0000x reference)
"""Coord2HeatmapNet Trainium2 kernel.

out[b,c,j,i] = 10*exp(-(((i+.5)/128 - x)^2 + ((j+.5)/128 - y)^2) / (2*(2/128)^2))

Exploited structure:
  * Separable: each heatmap = fy[j] (x) fx[i] outer product.
  * The gaussian has sigma = 2 px; beyond ~7 px the value is < 3.4e-3
    (vs peak 10.0), far below the 2e-2 relative tolerance. Only a
    WIN=16-row full-width window per heatmap is written; the pre-zeroed
    output buffer keeps the rest at 0.
  * The window is stored as fp16 on device (values in [0,10]; storage
    error <= 2^-11 of value, ~5e-4 of peak) and upcast to fp32 on the
    host after the gather. This halves the HBM write traffic, which is
    the roofline for this kernel.
  * Derivative_Erf activation = 2/sqrt(pi)*exp(-t^2): one ScalarE op per
    gaussian factor vector.
  * Layout: one heatmap per PARTITION. Partition p of group g holds
    half-windows (8 rows x 128) of heatmap k=g*128+p contiguously.
    The outer product is one DVE tensor_tensor per half with stride-0
    broadcasts; the write-out is one indirect scatter DMA per half
    (one offset per partition, data-dependent window position).
    Half-granularity lets the first scatter start ~1us after the first
    activation and keeps the SWDGE descriptor ring smoothly fed.
  * Each scatter group writes its own DRAM tensor: disjoint outputs let
    Tile overlap scatters instead of serializing on a WAW hazard it
    cannot disprove with data-dependent offsets.
  * Per-heatmap scalar metadata (activation biases, scatter offsets =
    affine transforms of coords; 544 values per core) is precomputed
    host-side and DMA'd in as small tables, removing the serial
    on-device preamble. Iota ramps are uploaded as constants.

Sharding: pure data parallel, 8 batches per core across 8 NeuronCores.
"""
import sys

for _p in ("/opt/trn_rl_repo", "/root/.axon_site", "/root/.axon_site/_ro/trn_rl_repo",
           "/root/.axon_site/_ro/pypackages"):
    if _p not in sys.path:
        sys.path.append(_p)

import numpy as np

S = 128
NUM_CLASS = 68
B_TOTAL = 64
N_CORES = 8
B_LOC = B_TOTAL // N_CORES            # 8 batches per core
NHM = B_LOC * NUM_CLASS               # 544 heatmaps per core
WIN = 16                              # window rows per heatmap
HWIN = WIN // 2                       # rows per scatter half
NG_FULL = NHM // 128                  # 4 full groups of 128 heatmaps
NG_REM = NHM - NG_FULL * 128          # 32 in the last group
GROUPS = [128] * NG_FULL + ([NG_REM] if NG_REM else [])
NG = len(GROUPS)
HFREE = HWIN * S                      # elems per scatter half
SIGMA = 2.0 / S
DENOM = 2.0 * SIGMA * SIGMA           # 1/2048
SINV = float(np.sqrt(1.0 / DENOM))    # 45.254834
A = SINV / S
AMP = float(10.0 * np.pi / 4.0)       # D_ERF = 2/sqrt(pi)*exp(-t^2); AMP*(2/sqrt(pi))^2 = 10
TCOLS = S + WIN + 2 * NG              # iota_i | riota | bx | by

_cache = {}


def _build():
    import concourse.bass as bass
    import concourse.tile as tile
    from concourse import bacc, mybir
    from concourse.bass import IndirectOffsetOnAxis
    from concourse.bass_types import AP

    f32 = mybir.dt.float32
    f16 = mybir.dt.float16
    i32 = mybir.dt.int32
    nc = bacc.Bacc("TRN2", target_bir_lowering=False, debug=False,
                   num_devices=N_CORES)

    tf_in = nc.dram_tensor("tf32", [128, TCOLS], f32, kind="ExternalInput")
    ti_in = nc.dram_tensor("ti32", [128, 2 * NG], i32, kind="ExternalInput")
    # One fp16 output tensor per scatter group (disjoint heatmap ranges).
    outs = [nc.dram_tensor(f"out{g}", [GROUPS[g] * S * S], f16,
                           kind="ExternalOutput") for g in range(NG)]
    o2ds = [o.ap().rearrange("(a b) -> a b", b=1) for o in outs]
    scratch = nc.dram_tensor("scr", [4], f16, kind="Internal")
    s2d = scratch.ap().rearrange("(a b) -> a b", b=1)

    derf = mybir.ActivationFunctionType.Derivative_Erf
    op = mybir.AluOpType

    with tile.TileContext(nc) as tc:
        with tc.tile_pool(name="tabs", bufs=1) as tp, \
             tc.tile_pool(name="main", bufs=10) as mp, \
             tc.tile_pool(name="vecs", bufs=5) as vp:
            # warm-ups while the table DMAs are in flight:
            # D_ERF table load on ScalarE (same bias/scale form as the
            # real activations, to avoid a second ucode table load) and
            # the SWDGE indirect path on Q7 (scatter into DRAM scratch,
            # no hazard on the real outputs).
            WZ = tp.tile([2, 2], f16)
            nc.gpsimd.iota(WZ[:], pattern=[[0, 2]], base=0,
                           channel_multiplier=0,
                           allow_small_or_imprecise_dtypes=True)
            WZB = tp.tile([2, 1], f32)
            nc.gpsimd.iota(WZB[:], pattern=[[0, 1]], base=0,
                           channel_multiplier=0,
                           allow_small_or_imprecise_dtypes=True)
            warm = tp.tile([2, 2], f16)
            nc.scalar.activation(warm[:], WZ[:], derf,
                                 bias=WZB[:], scale=A)
            WOFF = tp.tile([2, 1], i32)
            nc.gpsimd.iota(WOFF[:], pattern=[[1, 1]], base=0,
                           channel_multiplier=1,
                           allow_small_or_imprecise_dtypes=True)
            nc.gpsimd.indirect_dma_start(
                s2d, IndirectOffsetOnAxis(ap=WOFF[:], axis=0),
                WZ[:, 0:1], None)

            # host-precomputed tables (two parallel HWDGE rings)
            TF = tp.tile([128, TCOLS], f32)
            nc.sync.dma_start(TF[:], tf_in.ap())
            TI = tp.tile([128, 2 * NG], i32)
            nc.scalar.dma_start(TI[:], ti_in.ap())

            # ---- main loop: one group of <=128 heatmaps per iteration ----
            order = ([NG - 1] if NG_REM else []) + list(range(NG_FULL))
            for g in order:
                n = GROUPS[g]
                FY = vp.tile([128, WIN], f32, tag="fy")    # fy row per hm
                nc.scalar.activation(FY[0:n, :], TF[0:n, S:S + WIN], derf,
                                     bias=TF[0:n, S + WIN + NG + g:S + WIN + NG + g + 1],
                                     scale=A)
                FX = vp.tile([128, S], f16, tag="fx")      # fx row per hm
                nc.scalar.activation(FX[0:n, :], TF[0:n, 0:S], derf,
                                     bias=TF[0:n, S + WIN + g:S + WIN + g + 1],
                                     scale=A)
                FYA = vp.tile([128, WIN], f16, tag="fya")  # AMP*fy, fp16
                nc.vector.tensor_scalar_mul(FYA[0:n, :], FY[0:n, :], AMP)

                fxap = FX[0:n, :]
                for h in range(2):
                    fyap = FYA[0:n, h * HWIN:(h + 1) * HWIN]
                    G = mp.tile([128, HFREE], f16, tag=f"g{h}")
                    in0 = AP(tensor=fyap.tensor, offset=fyap.offset,
                             ap=[[fyap.ap[0][0], n], [1, HWIN], [0, S]])
                    in1 = AP(tensor=fxap.tensor, offset=fxap.offset,
                             ap=[[fxap.ap[0][0], n], [0, HWIN], [1, S]])
                    nc.vector.tensor_tensor(G[0:n, :], in0, in1, op.mult)
                    nc.gpsimd.indirect_dma_start(
                        o2ds[g],
                        IndirectOffsetOnAxis(
                            ap=TI[0:n, 2 * g + h:2 * g + h + 1], axis=0),
                        G[0:n, :], None)

    nc.compile()
    return nc


def _get_nc():
    if "nc" not in _cache:
        _cache["nc"] = _build()
    return _cache["nc"]


def _make_tables(coords_loc):
    """Per-heatmap activation biases and scatter offsets (host side).

    k = b*68 + c -> x = coords[b, 2c], y = coords[b, 2c+1]
      fx[i] = D_ERF(A*i + bx),  bx = A/2 - SINV*x
      fy[r] = D_ERF(A*r + by),  by = A*(jo + 0.5) - SINV*y
      jo    = clip(rint(128*y) - WIN/2, 0, 128-WIN)
      off   = (k%128)*128*128 + jo*128   (elements, local to the
              group's own output tensor; second scatter half +HWIN*128)
    """
    c3 = coords_loc.reshape(NHM, 2)
    x = c3[:, 0].astype(np.float64)
    y = c3[:, 1].astype(np.float64)
    jo = np.clip(np.rint(S * y) - WIN // 2, 0, S - WIN)
    bx = A * 0.5 - SINV * x
    by = A * (jo + 0.5) - SINV * y
    kloc = np.arange(NHM) % 128
    off = (kloc * S * S + jo * S).astype(np.int32)

    PAD = NG * 128
    bxp = np.zeros(PAD, np.float32); bxp[:NHM] = bx
    byp = np.zeros(PAD, np.float32); byp[:NHM] = by
    offp = np.zeros(PAD, np.int32);  offp[:NHM] = off

    tf = np.empty((128, TCOLS), np.float32)
    tf[:, 0:S] = np.arange(S, dtype=np.float32)[None, :]
    tf[:, S:S + WIN] = np.arange(WIN, dtype=np.float32)[None, :]
    tf[:, S + WIN:S + WIN + NG] = bxp.reshape(NG, 128).T
    tf[:, S + WIN + NG:] = byp.reshape(NG, 128).T
    og = offp.reshape(NG, 128).T               # [128, NG]
    ti = np.empty((128, 2 * NG), np.int32)
    ti[:, 0::2] = og
    ti[:, 1::2] = og + HWIN * S
    return tf, ti


def _run(coords_full, trace=False):
    from concourse.bass_utils import run_bass_kernel_spmd

    coords_full = np.ascontiguousarray(np.asarray(coords_full, dtype=np.float32))
    assert coords_full.shape == (B_TOTAL, 2 * NUM_CLASS)
    nc = _get_nc()
    in_maps = []
    for i in range(N_CORES):
        tf, ti = _make_tables(coords_full[i * B_LOC:(i + 1) * B_LOC])
        in_maps.append({"tf32": tf, "ti32": ti})
    br = run_bass_kernel_spmd(nc, in_maps, core_ids=list(range(N_CORES)),
                              trace=trace)
    parts = [
        np.concatenate([br.results[i][f"out{g}"].astype(np.float32)
                        for g in range(NG)])
        .reshape(B_LOC, NUM_CLASS, S, S)
        for i in range(N_CORES)
    ]
    full = np.concatenate(parts, axis=0)
    return full, br


def kernel(coords):
    return _run(coords, trace=False)[0]


# revision 13
# speedup vs baseline: 1.3795x; 1.3795x over previous
"""Coord2HeatmapNet Trainium2 kernel.

out[b,c,j,i] = 10*exp(-(((i+.5)/128 - x)^2 + ((j+.5)/128 - y)^2) / (2*(2/128)^2))

Exploited structure:
  * Separable: each heatmap = fy[j] (x) fx[i] outer product.
  * The gaussian has sigma = 2 px; beyond ~7 px the value is < 3.4e-3
    (vs peak 10.0), far below the 2e-2 relative tolerance. Only a
    WIN=16-row full-width window per heatmap is written; the pre-zeroed
    output buffer keeps the rest at 0.
  * The window is stored as fp16 on device (values in [0,10]; storage
    error <= 2^-11 of value, ~5e-4 of peak) and upcast to fp32 on the
    host after the gather. This halves the HBM write traffic, which is
    the roofline for this kernel.
  * Derivative_Erf activation = 2/sqrt(pi)*exp(-t^2): one ScalarE op per
    gaussian factor vector.
  * Layout: one heatmap per PARTITION. Partition p of group g holds
    half-windows (8 rows x 128) of heatmap k=g*128+p contiguously.
    The outer product is one DVE tensor_tensor per half with stride-0
    broadcasts; the write-out is one indirect scatter DMA per half
    (one offset per partition, data-dependent window position).
    Half-granularity lets the first scatter start ~1us after the first
    activation and keeps the SWDGE descriptor ring smoothly fed.
  * Each scatter group writes its own DRAM tensor: disjoint outputs let
    Tile overlap scatters instead of serializing on a WAW hazard it
    cannot disprove with data-dependent offsets.
  * Per-heatmap scalar metadata (activation biases, scatter offsets =
    affine transforms of coords; 544 values per core) is precomputed
    host-side and DMA'd in as small tables, removing the serial
    on-device preamble. Iota ramps are uploaded as constants.

Sharding: pure data parallel, 8 batches per core across 8 NeuronCores.
"""
import sys

for _p in ("/opt/trn_rl_repo", "/root/.axon_site", "/root/.axon_site/_ro/trn_rl_repo",
           "/root/.axon_site/_ro/pypackages"):
    if _p not in sys.path:
        sys.path.append(_p)

import numpy as np

S = 128
NUM_CLASS = 68
B_TOTAL = 64
N_CORES = 8
B_LOC = B_TOTAL // N_CORES            # 8 batches per core
NHM = B_LOC * NUM_CLASS               # 544 heatmaps per core
WIN = 16                              # window rows per heatmap
HWIN = WIN // 2                       # rows per scatter half
NG_FULL = NHM // 128                  # 4 full groups of 128 heatmaps
NG_REM = NHM - NG_FULL * 128          # 32 in the last group
GROUPS = [128] * NG_FULL + ([NG_REM] if NG_REM else [])
NG = len(GROUPS)
HFREE = HWIN * S                      # elems per scatter half
SIGMA = 2.0 / S
DENOM = 2.0 * SIGMA * SIGMA           # 1/2048
SINV = float(np.sqrt(1.0 / DENOM))    # 45.254834
A = SINV / S
AMP = float(10.0 * np.pi / 4.0)       # D_ERF = 2/sqrt(pi)*exp(-t^2); AMP*(2/sqrt(pi))^2 = 10
TCOLS = S + WIN + 2 * NG              # iota_i | riota | bx | by

_cache = {}


def _build():
    import concourse.bass as bass
    import concourse.tile as tile
    from concourse import bacc, mybir
    from concourse.bass import IndirectOffsetOnAxis
    from concourse.bass_types import AP

    f32 = mybir.dt.float32
    f16 = mybir.dt.float16
    i32 = mybir.dt.int32
    nc = bacc.Bacc("TRN2", target_bir_lowering=False, debug=False,
                   num_devices=N_CORES)

    tf_in = nc.dram_tensor("tf32", [128, TCOLS], f32, kind="ExternalInput")
    ti_in = nc.dram_tensor("ti32", [128, NG], i32, kind="ExternalInput")
    # One fp16 output tensor per scatter group (disjoint heatmap ranges).
    outs = [nc.dram_tensor(f"out{g}", [GROUPS[g] * S * S], f16,
                           kind="ExternalOutput") for g in range(NG)]
    o2ds = [o.ap().rearrange("(a b) -> a b", b=1) for o in outs]
    scratch = nc.dram_tensor("scr", [4], f16, kind="Internal")
    s2d = scratch.ap().rearrange("(a b) -> a b", b=1)

    derf = mybir.ActivationFunctionType.Derivative_Erf
    op = mybir.AluOpType

    with tile.TileContext(nc) as tc:
        with tc.tile_pool(name="tabs", bufs=1) as tp, \
             tc.tile_pool(name="main", bufs=5) as mp, \
             tc.tile_pool(name="vecs", bufs=5) as vp:
            # warm-ups while the table DMAs are in flight:
            # D_ERF table load on ScalarE (same bias/scale form as the
            # real activations, to avoid a second ucode table load) and
            # the SWDGE indirect path on Q7 (scatter into DRAM scratch,
            # no hazard on the real outputs).
            WZ = tp.tile([2, 2], f16)
            nc.gpsimd.iota(WZ[:], pattern=[[0, 2]], base=0,
                           channel_multiplier=0,
                           allow_small_or_imprecise_dtypes=True)
            WZB = tp.tile([2, 1], f32)
            nc.gpsimd.iota(WZB[:], pattern=[[0, 1]], base=0,
                           channel_multiplier=0,
                           allow_small_or_imprecise_dtypes=True)
            warm = tp.tile([2, 2], f16)
            nc.scalar.activation(warm[:], WZ[:], derf,
                                 bias=WZB[:], scale=A)
            WOFF = tp.tile([2, 1], i32)
            nc.gpsimd.iota(WOFF[:], pattern=[[1, 1]], base=0,
                           channel_multiplier=1,
                           allow_small_or_imprecise_dtypes=True)
            nc.gpsimd.indirect_dma_start(
                s2d, IndirectOffsetOnAxis(ap=WOFF[:], axis=0),
                WZ[:, 0:1], None)

            # host-precomputed tables (two parallel HWDGE rings)
            TF = tp.tile([128, TCOLS], f32)
            nc.sync.dma_start(TF[:], tf_in.ap())
            TI = tp.tile([128, NG], i32)
            nc.scalar.dma_start(TI[:], ti_in.ap())

            # ---- main loop: one group of <=128 heatmaps per iteration ----
            order = ([NG - 1] if NG_REM else []) + list(range(NG_FULL))
            for g in order:
                n = GROUPS[g]
                FY = vp.tile([128, WIN], f16, tag="fy")    # fy row per hm
                nc.scalar.activation(FY[0:n, :], TF[0:n, S:S + WIN], derf,
                                     bias=TF[0:n, S + WIN + NG + g:S + WIN + NG + g + 1],
                                     scale=A)
                FX = vp.tile([128, S], f16, tag="fx")      # fx row per hm
                nc.scalar.activation(FX[0:n, :], TF[0:n, 0:S], derf,
                                     bias=TF[0:n, S + WIN + g:S + WIN + g + 1],
                                     scale=A)
                FYA = vp.tile([128, WIN], f16, tag="fya")  # AMP*fy
                nc.vector.tensor_scalar_mul(FYA[0:n, :], FY[0:n, :], AMP)

                fxap = FX[0:n, :]
                fyap = FYA[0:n, :]
                G = mp.tile([128, WIN * S], f16, tag="g")
                in0 = AP(tensor=fyap.tensor, offset=fyap.offset,
                         ap=[[fyap.ap[0][0], n], [1, WIN], [0, S]])
                in1 = AP(tensor=fxap.tensor, offset=fxap.offset,
                         ap=[[fxap.ap[0][0], n], [0, WIN], [1, S]])
                nc.vector.tensor_tensor(G[0:n, :], in0, in1, op.mult)
                nc.gpsimd.indirect_dma_start(
                    o2ds[g],
                    IndirectOffsetOnAxis(ap=TI[0:n, g:g + 1], axis=0),
                    G[0:n, :], None)

    nc.compile()
    return nc


def _get_nc():
    if "nc" not in _cache:
        _cache["nc"] = _build()
    return _cache["nc"]


def _make_tables(coords_loc):
    """Per-heatmap activation biases and scatter offsets (host side).

    k = b*68 + c -> x = coords[b, 2c], y = coords[b, 2c+1]
      fx[i] = D_ERF(A*i + bx),  bx = A/2 - SINV*x
      fy[r] = D_ERF(A*r + by),  by = A*(jo + 0.5) - SINV*y
      jo    = clip(rint(128*y) - WIN/2, 0, 128-WIN)
      off   = (k%128)*128*128 + jo*128   (elements, local to the
              group's own output tensor; second scatter half +HWIN*128)
    """
    c3 = coords_loc.reshape(NHM, 2)
    x = c3[:, 0].astype(np.float64)
    y = c3[:, 1].astype(np.float64)
    jo = np.clip(np.rint(S * y) - WIN // 2, 0, S - WIN)
    bx = A * 0.5 - SINV * x
    by = A * (jo + 0.5) - SINV * y
    kloc = np.arange(NHM) % 128
    off = (kloc * S * S + jo * S).astype(np.int32)

    PAD = NG * 128
    bxp = np.zeros(PAD, np.float32); bxp[:NHM] = bx
    byp = np.zeros(PAD, np.float32); byp[:NHM] = by
    offp = np.zeros(PAD, np.int32);  offp[:NHM] = off

    tf = np.empty((128, TCOLS), np.float32)
    tf[:, 0:S] = np.arange(S, dtype=np.float32)[None, :]
    tf[:, S:S + WIN] = np.arange(WIN, dtype=np.float32)[None, :]
    tf[:, S + WIN:S + WIN + NG] = bxp.reshape(NG, 128).T
    tf[:, S + WIN + NG:] = byp.reshape(NG, 128).T
    ti = np.ascontiguousarray(offp.reshape(NG, 128).T)   # [128, NG]
    return tf, ti


def _run(coords_full, trace=False):
    from concourse.bass_utils import run_bass_kernel_spmd

    coords_full = np.ascontiguousarray(np.asarray(coords_full, dtype=np.float32))
    assert coords_full.shape == (B_TOTAL, 2 * NUM_CLASS)
    nc = _get_nc()
    in_maps = []
    for i in range(N_CORES):
        tf, ti = _make_tables(coords_full[i * B_LOC:(i + 1) * B_LOC])
        in_maps.append({"tf32": tf, "ti32": ti})
    br = run_bass_kernel_spmd(nc, in_maps, core_ids=list(range(N_CORES)),
                              trace=trace)
    parts = [
        np.concatenate([br.results[i][f"out{g}"].astype(np.float32)
                        for g in range(NG)])
        .reshape(B_LOC, NUM_CLASS, S, S)
        for i in range(N_CORES)
    ]
    full = np.concatenate(parts, axis=0)
    return full, br


def kernel(coords):
    return _run(coords, trace=False)[0]
